# revision 7
# baseline (speedup 1.0000x reference)
"""Trainium2 Bass kernel for nn_ApproxCompressor (v2, merged layout).

Reference (per sample n):
    alpha = sigmoid(z_alpha); h[k] = (1-alpha)*alpha^k (k<16384)
    env   = causal_conv(mean_c x^2, h); LG = log(env + 1e-5)
    quadratic-knee gain; out = gain * x.

Kernel strategy (8 cores x 4 samples):
  * Layout: all 4 local samples in one (128, F) tile set — sample s owns
    partitions [32s, 32s+32); each partition holds a contiguous 4096-sample
    signal chunk.  Per-sample scalars are (128,1) columns (ACT bias/scale,
    tensor_scalar operands), so ONE instruction per stage covers all samples.
  * The 16K-tap exponential FIR == one-pole IIR y[t] = a*y[t-1] + D[t].
    DVE tensor_tensor_scan runs the recurrence per partition (data0 is a
    stride-0 broadcast of the alpha column — HW-validated exact); free-dim
    chunked scans are chained via `initial`.  Cross-partition carries are
    fixed exactly: a block-diagonal 128x128 decay matmul (PE) produces the
    per-partition initial states, applied to the first JF columns via one
    scalar_tensor_tensor with a host-precomputed power table.
  * Quadratic knee, branch-free and cancellation-free (f32):
        A = relu(d+W); C = min(A, 2W); log_gain = c4w * C * (2A - C)
  * bf16 for the energy/gain-apply path (input cast during SWDGE DMA with
    big descriptors; bf16 TT runs at DVE 2x), f32 for decay + knee.
"""

import os
import sys

import numpy as np


def _import_concourse():
    try:
        import concourse.bass  # noqa: F401
    except ImportError:
        for p in ("/opt/trn_rl_repo", "/root/.axon_site/_ro/trn_rl_repo"):
            if os.path.isdir(p) and p not in sys.path:
                sys.path.insert(0, p)
        import concourse.bass  # noqa: F401


_import_concourse()

import concourse.bass as bass  # noqa: E402
import concourse.tile as tile  # noqa: E402
from concourse import bacc, mybir  # noqa: E402

N, C, L = 32, 2, 131072
NCORES = 8
NLOC = N // NCORES  # 4 samples per core
P = 128
SPP = P // NLOC  # 32 partitions per sample
FCH = L // SPP  # 4096 signal samples per partition row
NCH = 2  # free-dim pipeline chunks
W_CH = FCH // NCH
JF = 256  # carry-fix reach (alpha^JF below underflow for all plausible alpha)
EPS = 1e-5
K_FIR = 16384

F32 = mybir.dt.float32
BF16 = mybir.dt.bfloat16

PRM_ALPHA, PRM_LNSCALE, PRM_B1, PRM_W2, PRM_C4W, PRM_EPS = 0, 1, 2, 3, 4, 5
NPRM = 8

ACT_SET_ID = 6  # natural_log_exp_and_others: ln, exp, relu, square

# engine per op: 's'calar(ACT) / 'v'ector(DVE) / 'g'psimd
ENG = {
    "sq0": "s",
    "sq1": "s",
    "eadd": "g",
    "C": "v",
    "Z": "v",
    "Q": "v",
    "out0": "v",
    "out1": "g",
}
_E = {"s": "scalar", "v": "vector", "g": "gpsimd"}

TRACE_RESULT = {}


def _eng(nc, name):
    return getattr(nc, _E[ENG[name]])


def _bcast(col_ap, n):
    """Free-dim stride-0 broadcast of a (128,1) column AP to (128,n)."""
    return bass.AP(col_ap.tensor, col_ap.offset, [list(col_ap.ap[0]), [0, n]])


def build_nc():
    AF = mybir.ActivationFunctionType
    OP = mybir.AluOpType

    nc = bacc.Bacc(
        "TRN2", target_bir_lowering=False, num_devices=NCORES, num_swdge_queues=4
    )
    x_ext = nc.declare_dram_parameter("x", [NLOC, C, L], F32, isOutput=False)
    prm_ext = nc.declare_dram_parameter("prm", [P, NPRM], F32, isOutput=False)
    tri_ext = nc.declare_dram_parameter("tri", [P, P], F32, isOutput=False)
    pw_ext = nc.declare_dram_parameter("pw", [P, JF], F32, isOutput=False)
    out_ext = nc.declare_dram_parameter("out", [NLOC, C, L], BF16, isOutput=True)

    # DRAM views with partition split (s, q): addr = s*C*L + ch*L + q*FCH + t
    x4 = [x_ext[:, ch].rearrange("s (q t) -> s q t", q=SPP) for ch in range(C)]
    o4 = [out_ext[:, ch].rearrange("s (q t) -> s q t", q=SPP) for ch in range(C)]

    with tile.TileContext(nc) as tc:
        atl = mybir.InstLoadActFuncSet(
            name=nc.get_next_instruction_name(), ins=[], outs=[],
            act_func_set_id=ACT_SET_ID,
        )
        nc.scalar.add_instruction(atl)
        with (
            tc.tile_pool(name="pc", bufs=1) as pc,
            tc.tile_pool(name="pbig", bufs=1) as pbig,
            tc.tile_pool(name="py", bufs=NCH + 1) as py,
            tc.tile_pool(name="pw2", bufs=2) as pw2,
            tc.tile_pool(name="pps", bufs=2, space=bass.MemorySpace.PSUM) as pps,
        ):
            prm = pc.tile([P, NPRM], F32, tag="prm")
            nc.sync.dma_start(out=prm[:], in_=prm_ext[:])
            tri = pc.tile([P, P], F32, tag="tri")
            nc.sync.dma_start(out=tri[:], in_=tri_ext[:])
            pw = pc.tile([P, JF], F32, tag="pw")
            nc.sync.dma_start(out=pw[:], in_=pw_ext[:])

            a_col = prm[:, PRM_ALPHA : PRM_ALPHA + 1]
            lnscale_col = prm[:, PRM_LNSCALE : PRM_LNSCALE + 1]
            b1_col = prm[:, PRM_B1 : PRM_B1 + 1]
            w2_col = prm[:, PRM_W2 : PRM_W2 + 1]
            c4w_col = prm[:, PRM_C4W : PRM_C4W + 1]
            eps_col = prm[:, PRM_EPS : PRM_EPS + 1]

            # ---- input: SWDGE cast DMA per chunk (big descriptors) ---------
            xb = pbig.tile([P, C * FCH], BF16, tag="xb")
            for k in range(NCH):
                for ch in range(C):
                    nc.gpsimd.dma_start(
                        out=xb[:, ch * FCH + k * W_CH : ch * FCH + (k + 1) * W_CH],
                        in_=x4[ch][:, :, k * W_CH : (k + 1) * W_CH],
                    )

            # ---- energy + scans per chunk ----------------------------------
            y1s = []
            for k in range(NCH):
                x0k = xb[:, k * W_CH : (k + 1) * W_CH]
                x1k = xb[:, FCH + k * W_CH : FCH + (k + 1) * W_CH]
                sq0 = pw2.tile([P, W_CH], BF16, tag="sq0")
                if ENG["sq0"] == "s":
                    nc.scalar.activation(sq0[:], x0k, AF.Square)
                else:
                    _eng(nc, "sq0").tensor_tensor(sq0[:], x0k, x0k, OP.mult)
                sq1 = pw2.tile([P, W_CH], BF16, tag="sq1")
                if ENG["sq1"] == "s":
                    nc.scalar.activation(sq1[:], x1k, AF.Square)
                else:
                    _eng(nc, "sq1").tensor_tensor(sq1[:], x1k, x1k, OP.mult)
                D = pw2.tile([P, W_CH], BF16, tag="D")
                _eng(nc, "eadd").tensor_tensor(D[:], sq0[:], sq1[:], OP.add)

                y1 = py.tile([P, W_CH], F32, tag="y1")
                init = 0.0 if k == 0 else y1s[k - 1][:, W_CH - 1 : W_CH]
                nc.vector.tensor_tensor_scan(
                    y1[:], _bcast(a_col, W_CH), D[:], init, OP.mult, OP.add
                )
                y1s.append(y1)

            # ---- cross-partition carry fix ---------------------------------
            s_col = pps.tile([P, 1], F32, tag="s_col")
            nc.tensor.matmul(
                s_col[:], tri[:], y1s[NCH - 1][:, W_CH - 1 : W_CH],
                start=True, stop=True,
            )
            # y1_0[:, :JF] += s * pw   (in place)
            nc.vector.scalar_tensor_tensor(
                y1s[0][:, 0:JF], pw[:], s_col[:, 0:1], y1s[0][:, 0:JF],
                OP.mult, OP.add,
            )

            # ---- gain computer + apply, chunk 0 last (waits for the fix) ---
            for k in list(range(1, NCH)) + [0]:
                y1 = y1s[k]
                LG = pw2.tile([P, W_CH], F32, tag="LG")
                nc.scalar.activation(
                    LG[:], y1[:], AF.Ln, bias=eps_col, scale=lnscale_col
                )
                A = pw2.tile([P, W_CH], F32, tag="A")
                nc.scalar.activation(A[:], LG[:], AF.Relu, bias=b1_col)
                Ct = pw2.tile([P, W_CH], F32, tag="LG")  # share slots with LG
                _eng(nc, "C").tensor_scalar_min(Ct[:], A[:], w2_col)
                Z = pw2.tile([P, W_CH], F32, tag="A")  # share slots with A
                _eng(nc, "Z").scalar_tensor_tensor(
                    Z[:], A[:], 2.0, Ct[:], OP.mult, OP.subtract
                )
                Q = pw2.tile([P, W_CH], F32, tag="Q")
                _eng(nc, "Q").tensor_tensor(Q[:], Ct[:], Z[:], OP.mult)
                gain = pw2.tile([P, W_CH], BF16, tag="gain")
                nc.scalar.activation(gain[:], Q[:], AF.Exp, scale=c4w_col)

                ot = pw2.tile([P, C * W_CH], BF16, tag="ot")
                _eng(nc, "out0").tensor_tensor(
                    ot[:, 0:W_CH], gain[:], xb[:, k * W_CH : (k + 1) * W_CH],
                    OP.mult,
                )
                _eng(nc, "out1").tensor_tensor(
                    ot[:, W_CH : 2 * W_CH], gain[:],
                    xb[:, FCH + k * W_CH : FCH + (k + 1) * W_CH], OP.mult,
                )
                for ch in range(C):
                    nc.sync.dma_start(
                        out=o4[ch][:, :, k * W_CH : (k + 1) * W_CH],
                        in_=ot[:, ch * W_CH : (ch + 1) * W_CH],
                    )
    nc.finalize()
    return nc


def host_params(z_alpha, log_threshold, log_ratio, log_knee):
    """Per-core param/tri/pw tensors, float32 (scalars via float64 math)."""
    z = z_alpha.astype(np.float64).reshape(-1)
    alpha = 1.0 / (1.0 + np.exp(-z))
    aK = np.exp(K_FIR * np.log(alpha))
    assert np.all(aK < 1e-6), "FIR tail non-negligible; needs shift correction"
    aJ = np.exp(JF * np.log(alpha))
    assert np.all(aJ < 1e-7), "carry-fix reach JF too small for this alpha"
    T = log_threshold.astype(np.float64).reshape(-1) - 6.0
    R = 1.0 + np.exp(log_ratio.astype(np.float64).reshape(-1))
    W = np.exp(log_knee.astype(np.float64).reshape(-1))
    c = 1.0 / R - 1.0

    n = alpha.shape[0]
    prms, tris, pws = [], [], []
    j = np.arange(1, JF + 1, dtype=np.float64)
    kq = np.arange(SPP)[None, :] - 1 - np.arange(SPP)[:, None]  # [q_src, q_dst]
    for c0 in range(n // NLOC):
        sl = slice(c0 * NLOC, (c0 + 1) * NLOC)
        a4, T4, W4, c4 = alpha[sl], T[sl], W[sl], c[sl]
        prm = np.zeros((P, NPRM), np.float64)
        rep = np.repeat
        prm[:, PRM_ALPHA] = rep(a4, SPP)
        prm[:, PRM_LNSCALE] = rep(0.5 * (1.0 - a4), SPP)
        prm[:, PRM_B1] = rep(W4 - T4, SPP)
        prm[:, PRM_W2] = rep(2.0 * W4, SPP)
        prm[:, PRM_C4W] = rep(c4 / (4.0 * W4), SPP)
        prm[:, PRM_EPS] = EPS
        prms.append(prm.astype(np.float32))

        tri = np.zeros((P, P), np.float64)
        pwm = np.zeros((P, JF), np.float64)
        for s in range(NLOC):
            expo = FCH * kq * np.log(a4[s])
            m = (kq >= 0) & (expo > -100.0)
            blk = np.zeros((SPP, SPP))
            blk[m] = np.exp(expo[m])
            tri[s * SPP : (s + 1) * SPP, s * SPP : (s + 1) * SPP] = blk
            pwm[s * SPP : (s + 1) * SPP, :] = np.exp(j * np.log(a4[s]))[None, :]
        tris.append(tri.astype(np.float32))
        pws.append(pwm.astype(np.float32))
    return prms, tris, pws


def _ensure_ntff_hook():
    import types

    try:
        from antenv.axon_hooks import get_axon_ntff_profile_hook  # noqa: F401

        return
    except ImportError:
        pass
    try:
        from trn_agent_boot.trn_boot import _ntff_profile_via_ctypes
    except ImportError:
        return
    hook = _ntff_profile_via_ctypes("/opt/axon/libaxon_pjrt.so")
    mod = types.ModuleType("antenv.axon_hooks")
    mod._hook = hook
    mod.get_axon_ntff_profile_hook = lambda: mod._hook

    def set_axon_ntff_profile_hook(h):
        mod._hook = h

    mod.set_axon_ntff_profile_hook = set_axon_ntff_profile_hook
    import antenv

    sys.modules["antenv.axon_hooks"] = mod
    antenv.axon_hooks = mod


def kernel(input_signals, z_alpha, log_threshold, log_ratio, log_knee):
    from concourse.bass_utils import run_bass_kernel_spmd

    x = np.asarray(input_signals, np.float32)
    prms, tris, pws = host_params(
        np.asarray(z_alpha), np.asarray(log_threshold),
        np.asarray(log_ratio), np.asarray(log_knee),
    )

    nc = build_nc()
    core_ids = list(range(NCORES))
    in_maps = [
        {
            "x": np.ascontiguousarray(x[i * NLOC : (i + 1) * NLOC]),
            "prm": prms[i],
            "tri": tris[i],
            "pw": pws[i],
        }
        for i in core_ids
    ]

    trace = os.environ.get("BASS_KERNEL_TRACE", "0") == "1"
    if trace:
        _ensure_ntff_hook()
    res = run_bass_kernel_spmd(nc, in_maps, core_ids, trace=trace)
    if trace:
        TRACE_RESULT["exec_time_ns"] = res.exec_time_ns
        TRACE_RESULT["results"] = res

    out = np.empty((N, C, L), np.float32)
    for i in core_ids:
        o = np.asarray(res.results[i]["out"], np.float32)  # (NLOC, C, L)
        out[i * NLOC : (i + 1) * NLOC] = o
    return out
